# revision 44
# baseline (speedup 1.0000x reference)
"""Trainium2 Bass kernel for causal multi-head attention (v3).

Problem: x[64,256,512] f32, Wq/Wk/Wv[8,512,64], Wo[512,512]
  q,k,v = einsum('btc,hcd->bhtd'); scores = q k^T / sqrt(512) (causal);
  out = softmax(scores) v; y = concat-heads(out) @ Wo.

Strategy: data-parallel over batch across 8 NeuronCores (8 batches/core,
no collectives), all-fp16 operands with fp32 PSUM accumulation.

v3 changes vs v2 (which used moving-P PV + reciprocal[1,512] + Pool
partition_broadcast + per-column normalize):
  - PV uses the masked P blocks as STATIONARY ([s,t] blocks) and V' as
    moving ([s, 65] with a ones column), producing out[t, h, 65] in PSUM
    with t on PARTITIONS; col 64 is the softmax denominator.
  - Normalize is then per-partition: one cheap [128,4] reciprocal and one
    broadcast tensor_mul per (tb, head-group) on the DVE, fusing the fp16
    downcast. No Pool broadcast, no 1-partition reciprocal.
  - out[t, hd] is transposed back to outT[hd, t] for the output
    projection with 4 PE transposes per (b, tb) (fp16 PSUM out) + 1 copy.
  - PSUM->SBUF copies distributed across Act/DVE/Pool engines (knobs).
  - y output DMAd as fp16 (upcast on host): halves output traffic.
"""
import numpy as np

import concourse.bass as bass
import concourse.tile as tile
import concourse.mybir as mybir
from concourse import bacc
from concourse.alu_op_type import AluOpType
from concourse.bass_utils import run_bass_kernel_spmd

F32 = mybir.dt.float32
F16 = mybir.dt.float16
BF16 = mybir.dt.bfloat16

N_CORES = 8
B, T, C = 64, 256, 512
H, DK = 8, 64
B_LOC = B // N_CORES        # 8 batches per core
N_HP = H // 2               # head pairs (2x64 packed on partitions)
N_CC = C // 128             # contraction chunks
SCALE = 1.0 / np.sqrt(np.float32(C))

_DT = {"fp32": F32, "fp16": F16, "bf16": BF16}


def build_nc(cfg, repeat=0, tune=None, bodies=1):
    """repeat=0: straight-line kernel. repeat=R>0: wrap the whole pipeline
    in a hardware For_i loop executed R times (for timing). bodies=N emits
    the pipeline N times inside the loop (probe for loop-boundary cost)."""
    tu = {"big": 2, "s": 2, "pv": 3, "tr": 1, "p": 6, "qk": 3, "vv": 2,
          "xt": 2, "ot": 2, "oT": 2, "rc": 4,
          "qkcop": "dve", "vcop": "act", "ycop": "act", "trcop": "dve",
          "mask_eng": "mix", "norm": "on", "mask": "on",
          "esplit": 0, "nd": 0, "xchunk": 0, "ymerge": 1}
    tu.update(tune or {})
    for k, v in (cfg or {}).items():
        if k.startswith("t_"):
            kk = k[2:]
            tu[kk] = int(v) if str(v).isdigit() else v
    proj_dt = _DT[cfg.get("proj", "fp16")]      # xT / Wq / Wk / Wv operands
    sc_dt = _DT[cfg.get("scores", "fp16")]      # QT / KT operands
    pv_dt = _DT[cfg.get("pv", "fp16")]          # P / V' operands
    op_dt = _DT[cfg.get("outproj", "fp16")]     # outT / Wo operands

    nc = bacc.Bacc("TRN2", target_bir_lowering=False, debug=False)

    xT_d = nc.dram_tensor("xT", [128, N_CC * B_LOC * T], proj_dt,
                          kind="ExternalInput").ap()
    wq_d = nc.dram_tensor("wq", [128, N_HP * N_CC * 128], proj_dt, kind="ExternalInput").ap()
    wk_d = nc.dram_tensor("wk", [128, N_HP * N_CC * 128], proj_dt, kind="ExternalInput").ap()
    wv_d = nc.dram_tensor("wv", [128, N_CC * C], proj_dt, kind="ExternalInput").ap()
    wo_d = nc.dram_tensor("wo", [128, N_CC * C], op_dt, kind="ExternalInput").ap()
    mask_d = nc.dram_tensor("mask", [128, 128], pv_dt, kind="ExternalInput").ap()
    id_d = nc.dram_tensor("ident", [128, 128], op_dt, kind="ExternalInput").ap()
    y_d = nc.dram_tensor("y", [B_LOC, T, C], F16, kind="ExternalOutput").ap()

    def copier(which):
        eng = tu[which]
        if eng == "act":
            return nc.scalar.copy
        if eng == "pool":
            return nc.gpsimd.tensor_copy
        return nc.vector.tensor_copy

    with tile.TileContext(nc) as tc:
        import contextlib
        ctx = contextlib.ExitStack()
        with ctx:
            const = ctx.enter_context(tc.tile_pool(name="const", bufs=1))
            xT_p = ctx.enter_context(tc.tile_pool(name="xT", bufs=tu["xt"]))
            qk_p = ctx.enter_context(tc.tile_pool(name="qk", bufs=tu["qk"]))
            vv_p = ctx.enter_context(tc.tile_pool(name="vv", bufs=tu["vv"]))
            p_p = ctx.enter_context(tc.tile_pool(name="pp", bufs=tu["p"]))
            rc_p = ctx.enter_context(tc.tile_pool(name="rc", bufs=tu["rc"]))
            ot_p = ctx.enter_context(tc.tile_pool(name="ot", bufs=tu["ot"]))
            oT_p = ctx.enter_context(tc.tile_pool(name="oT", bufs=tu["oT"]))
            y_p = ctx.enter_context(tc.tile_pool(name="yp", bufs=2))
            big_ps = ctx.enter_context(tc.tile_pool(name="big_ps", bufs=tu["big"], space="PSUM"))
            s_ps = ctx.enter_context(tc.tile_pool(name="s_ps", bufs=tu["s"], space="PSUM"))
            pv_ps = ctx.enter_context(tc.tile_pool(name="pv_ps", bufs=tu["pv"], space="PSUM"))
            tr_ps = big_ps if not tu["tr"] else ctx.enter_context(
                tc.tile_pool(name="tr_ps", bufs=tu["tr"], space="PSUM"))

            # persistent constants / weights; wq/wk first (gate QK(0) in
            # the single-shot path), then wv/mask, then late-need wo/ident
            wq = const.tile([128, N_HP, N_CC, 128], proj_dt)
            nc.sync.dma_start(wq[:], wq_d.rearrange("p (a b c) -> p a b c", a=N_HP, b=N_CC))
            wk = const.tile([128, N_HP, N_CC, 128], proj_dt)
            nc.sync.dma_start(wk[:], wk_d.rearrange("p (a b c) -> p a b c", a=N_HP, b=N_CC))
            wv = const.tile([128, N_CC, C], proj_dt)
            nc.sync.dma_start(wv[:], wv_d.rearrange("p (a b) -> p a b", a=N_CC))
            mask2 = const.tile([128, 2, 128], pv_dt)
            nc.sync.dma_start(mask2[:, 0, :], mask_d[:])
            nc.sync.dma_start(mask2[:, 1, :], mask_d[:])
            ident = const.tile([128, 128], op_dt)
            nc.sync.dma_start(ident[:], id_d[:])
            wo = const.tile([128, N_CC, C], op_dt)
            nc.sync.dma_start(wo[:], wo_d.rearrange("p (a b) -> p a b", a=N_CC))
            if tu["mask"] == "bias":
                # additive pre-exp causal mask, injected via PE: each diag
                # S block's accumulation group starts with ident.T @ mrow
                # (= -6e4 on masked, 0 on valid), then the score matmul
                # accumulates on top. exp output is already masked, so PV
                # no longer waits on a post-exp DVE mask multiply.
                mrow = const.tile([128, 128], pv_dt)
                nc.vector.tensor_scalar(
                    mrow[:], mask2[:, 0, :], 6.0e4, 6.0e4,
                    AluOpType.mult, AluOpType.subtract)

            lp = nc.allow_low_precision(
                reason="fp16 operands with fp32 PSUM accumulation; "
                       "rel-err budget 2e-2")

            def body():
                # ---- load xT for all batches: [128, cc, b, t]; chunked
                # along cc (contiguous 4KB/partition segments) so the first
                # QK accumulation starts after 1/4 of the data arrives and
                # the load pipelines across For_i iterations ----
                xT = xT_p.tile([128, N_CC, B_LOC, T], proj_dt, tag="xT")
                xT_src = xT_d.rearrange("p (a b t) -> p a b t", a=N_CC, b=B_LOC)
                if tu["xchunk"]:
                    for cc in range(N_CC):
                        nc.sync.dma_start(xT[:, cc, :, :], xT_src[:, cc, :, :])
                else:
                    nc.sync.dma_start(xT[:], xT_src)

                # Filler-queue software pipeline: attention head-steps of
                # batch b interleave ready PE chunk-work from the future
                # (QK of pair+1, V of b+1, OP of b-1) so the in-order PE
                # queue always has independent work during chain stalls.
                from collections import deque
                filler = deque()
                state = {}
                qk_tiles = {}
                vv_tiles = {}
                outT_tiles = {}

                def make_qk_fillers(pair):
                    qt = qk_p.tile([128, N_HP, 2, T], sc_dt, tag="qt",
                                   name=f"qt{pair}")
                    kt = qk_p.tile([128, N_HP, 2, T], sc_dt, tag="kt",
                                   name=f"kt{pair}")
                    qk_tiles[pair] = (qt, kt)
                    chunks = []
                    for (w, dst) in ((wq, qt), (wk, kt)):
                        for hp in range(N_HP):
                            def chunk(w=w, dst=dst, hp=hp, pair=pair):
                                ps = big_ps.tile([128, 512], F32, tag="big")
                                for cc in range(N_CC):
                                    nc.tensor.matmul(
                                        ps[:], w[:, hp, cc, :],
                                        xT[:, cc, 2 * pair:2 * pair + 2, :].rearrange("p a t -> p (a t)"),
                                        start=(cc == 0), stop=(cc == N_CC - 1))
                                dstap = dst[:, hp, :, :].rearrange("p a t -> p (a t)")
                                copier("qkcop")(dstap, ps[:])
                            chunks.append(chunk)
                    return chunks

                def make_v_fillers(b):
                    vvt = vv_p.tile([128, 2, H, DK + 1], pv_dt, tag="vv",
                                    name=f"vv{b}")
                    vv_tiles[b] = vvt
                    chunks = []
                    for sc in range(2):
                        def chunk(sc=sc, vvt=vvt, b=b):
                            if sc == 0:
                                nc.vector.memset(vvt[:, :, :, DK:DK + 1], 1.0)
                            ps = big_ps.tile([128, 512], F32, tag="big")
                            for cc in range(N_CC):
                                nc.tensor.matmul(
                                    ps[:], xT[:, cc, b, bass.ts(sc, 128)], wv[:, cc, :],
                                    start=(cc == 0), stop=(cc == N_CC - 1))
                            copier("vcop")(
                                vvt[:, sc, :, 0:DK],
                                ps[:].rearrange("p (h d) -> p h d", h=H))
                        chunks.append(chunk)
                    return chunks

                def make_op_fillers(b):
                    outT = outT_tiles[b]
                    chunks = []
                    if tu["ymerge"]:
                        yt = y_p.tile([128, 2, C], F16, tag="yt", name=f"yt{b}")
                    for tb in range(2):
                        def chunk(tb=tb, outT=outT, b=b):
                            ps = big_ps.tile([128, 512], F32, tag="big")
                            for cc in range(N_CC):
                                nc.tensor.matmul(
                                    ps[:], outT[:, cc, tb, :], wo[:, cc, :],
                                    start=(cc == 0), stop=(cc == N_CC - 1))
                            if tu["ymerge"]:
                                copier("ycop")(yt[:, tb, :], ps[:])
                                if tb == 1:
                                    nc.sync.dma_start(
                                        y_d[b].rearrange("(a q) c -> q a c", a=2),
                                        yt[:])
                            else:
                                yt1 = y_p.tile([128, C], F16, tag="yt")
                                copier("ycop")(yt1[:], ps[:])
                                nc.sync.dma_start(y_d[b, bass.ts(tb, 128), :], yt1[:])
                        chunks.append(chunk)
                    return chunks

                def emit_S(b2, h, qt, kt):
                    hp, lo = h // 2, (h % 2) * DK
                    qs = qt[lo:lo + DK, hp, b2, :]
                    ks = kt[lo:lo + DK, hp, b2, :]
                    sps = s_ps.tile([128, 384], F32, tag="s")
                    if tu["mask"] == "bias":
                        # col layout [tri0 | tri1 | full]; diag block groups
                        # start with the additive-mask matmul (consts only,
                        # so PE can run it before qs/ks are even ready)
                        nc.tensor.matmul(sps[:, 0:128], ident[:], mrow[:],
                                         start=True, stop=False)
                        nc.tensor.matmul(sps[:, 0:128], ks[:, 0:128],
                                         qs[:, 0:128], start=False, stop=True)
                        nc.tensor.matmul(sps[:, 128:256], ident[:], mrow[:],
                                         start=True, stop=False)
                        nc.tensor.matmul(sps[:, 128:256], ks[:, 128:256],
                                         qs[:, 128:256], start=False, stop=True)
                        nc.tensor.matmul(sps[:, 256:384], ks[:, 0:128],
                                         qs[:, 128:256], start=True, stop=True)
                    else:
                        nc.tensor.matmul(sps[:, 0:256], ks[:, 0:128], qs[:],
                                         start=True, stop=True)
                        nc.tensor.matmul(sps[:, 256:384], ks[:, 128:256],
                                         qs[:, 128:256], start=True, stop=True)
                    pt = p_p.tile([128, 3, 128], pv_dt, tag="p")
                    p = pt[:].rearrange("q a b -> q (a b)")
                    Exp = mybir.ActivationFunctionType.Exp

                    def domask(pm, m):
                        if tu["mask_eng"] == "pool" or (
                                tu["mask_eng"] == "mix" and h % 2 == 1):
                            nc.gpsimd.tensor_mul(pm, pm, m)
                        else:
                            nc.vector.tensor_mul(pm, pm, m)

                    if tu["esplit"]:
                        # diag blocks first: unblocks mask+PV(A,C) sooner
                        sps3 = sps[:].rearrange("p (a b) -> p a b", a=3)
                        nc.scalar.activation(pt[:, 0:3:2, :], sps3[:, 0:3:2, :],
                                             Exp, bias=0.0, scale=float(SCALE))
                        if tu["mask"] != "off":
                            domask(pt[:, 0:3:2, :], mask2[:])
                        nc.scalar.activation(pt[:, 1, :], sps[:, 128:256],
                                             Exp, bias=0.0, scale=float(SCALE))
                    else:
                        nc.scalar.activation(p, sps[:], Exp,
                                             bias=0.0, scale=float(SCALE))
                        if tu["mask"] == "on":
                            domask(pt[:, 0:3:2, :], mask2[:])
                    state[h] = {"pt": pt}

                def emit_PV_B(b, h, vvt, pvts):
                    # full (unmasked) block: depends on exp only — emitted
                    # one step after S so it never waits on the mask mult
                    pt = state[h]["pt"]
                    g, slot = h // 4, h % 4
                    if slot == 0:
                        pvts[g] = (
                            pv_ps.tile([128, 4, DK + 1], F32, tag="pv",
                                       name=f"pv{b}t0g{g}"),
                            pv_ps.tile([128, 4, DK + 1], F32, tag="pv",
                                       name=f"pv{b}t1g{g}"),
                        )
                    p0, p1 = pvts[g]
                    iB = 2 if tu["mask"] == "bias" else 1
                    nc.tensor.matmul(p1[:, slot, :], pt[:, iB, :], vvt[:, 0, h, :],
                                     start=True, stop=False)

                def emit_PV_AC(b, h, vvt, pvts):
                    # masked diag blocks: emitted a further step later so the
                    # DVE mask multiply has a full head-step of slack
                    pt = state.pop(h)["pt"]
                    g, slot = h // 4, h % 4
                    p0, p1 = pvts[g]
                    iC = 1 if tu["mask"] == "bias" else 2
                    nc.tensor.matmul(p0[:, slot, :], pt[:, 0, :], vvt[:, 0, h, :],
                                     start=True, stop=True)
                    nc.tensor.matmul(p1[:, slot, :], pt[:, iC, :], vvt[:, 1, h, :],
                                     start=False, stop=True)

                def emit_norm(b, g, outn, pvts):
                    for tb in range(2):
                        pv = pvts[g][tb]
                        if tu["norm"] == "on":
                            rec = rc_p.tile([128, 4, 1], pv_dt, tag="rec")
                            nc.vector.reciprocal(rec[:], pv[:, :, DK:DK + 1])
                            nc.vector.tensor_mul(
                                outn[:, tb, bass.ts(g, 4), :], pv[:, :, 0:DK],
                                rec[:].broadcast_to([128, 4, DK]))
                        else:  # timing ablation: skip normalize chain
                            nc.vector.tensor_copy(
                                outn[:, tb, bass.ts(g, 4), :], pv[:, :, 0:DK])

                def emit_tr(b, outn, trst, ks):
                    # transpose head-blocks ks (0,1 ready after g0 norm;
                    # 2,3 after g1) for both t-blocks into one fp16 psum
                    # tile (both tbs fit in a single bank)
                    if trst["tps"] is None:
                        trst["tps"] = tr_ps.tile(
                            [128, 2, N_CC, 128], op_dt,
                            tag="big" if tr_ps is big_ps else "tr",
                            name=f"tr{b}")
                    tps = trst["tps"]
                    on2 = outn[:].rearrange("p a b c -> p a (b c)")
                    for tb in range(2):
                        for k in ks:
                            nc.tensor.transpose(
                                tps[:, tb, k, :],
                                on2[:, tb, bass.ts(k, 128)], ident[:])

                def emit_trcopy(b, trst):
                    outT = oT_p.tile([128, N_CC, 2, 128], op_dt, tag="outT",
                                     name=f"ot{b}")
                    outT_tiles[b] = outT
                    copier("trcop")(
                        outT[:], trst["tps"][:].rearrange("p a b c -> p b a c"))

                # bootstrap: QK(0) and V(0) emitted inline
                for f in make_qk_fillers(0):
                    f()
                for f in make_v_fillers(0):
                    f()

                for pair in range(B_LOC // 2):
                    qt, kt = qk_tiles[pair]
                    for b2 in range(2):
                        b = 2 * pair + b2
                        outn = ot_p.tile([128, 2, H, DK], pv_dt, tag="outn",
                                         name=f"on{b}")
                        pvts = {}
                        trst = {"tps": None}
                        if b2 == 1 and pair + 1 < B_LOC // 2:
                            filler.extend(make_qk_fillers(pair + 1))
                        if b + 1 < B_LOC:
                            filler.extend(make_v_fillers(b + 1))
                        if b - 1 >= 0:
                            filler.extend(make_op_fillers(b - 1))

                        vvt = vv_tiles[b]
                        for h in range(H):
                            emit_S(b2, h, qt, kt)
                            if tu["nd"] == 0:
                                ndrain = 2 if len(filler) > H - h else 1
                            elif tu["nd"] == 1:
                                ndrain = 1
                            elif tu["nd"] == 2:
                                ndrain = 2
                            else:  # front-loaded
                                ndrain = 2 if h < 4 else 1
                            for _ in range(ndrain):
                                if filler:
                                    filler.popleft()()
                            if h >= 2:
                                emit_PV_AC(b, h - 2, vvt, pvts)
                            if h >= 1:
                                emit_PV_B(b, h - 1, vvt, pvts)
                            if h == 5:
                                emit_norm(b, 0, outn, pvts)
                            if h == 6:
                                emit_tr(b, outn, trst, (0, 1))
                        emit_PV_AC(b, H - 2, vvt, pvts)
                        emit_PV_B(b, H - 1, vvt, pvts)
                        emit_PV_AC(b, H - 1, vvt, pvts)
                        emit_norm(b, 1, outn, pvts)
                        emit_tr(b, outn, trst, (2, 3))
                        emit_trcopy(b, trst)
                while filler:
                    filler.popleft()()
                for f in make_op_fillers(B_LOC - 1):
                    f()

            with lp:
                if repeat:
                    with tc.For_i(0, repeat, 1):
                        for _ in range(bodies):
                            body()
                else:
                    for _ in range(bodies):
                        body()

    nc.compile()
    return nc


def _prep_inputs(x, Wq, Wk, Wv, Wo, cfg):
    """Host-side reshapes/casts. Returns per-core input maps."""
    import ml_dtypes

    def npdt(key):
        s = cfg.get(key, "fp16")
        return {"fp32": np.float32, "fp16": np.float16,
                "bf16": ml_dtypes.bfloat16}[s]

    proj_np, sc_np, pv_np, op_np = (npdt(k) for k in
                                    ("proj", "scores", "pv", "outproj"))

    # weights: head-pair stationary blocks [128c, hp, cc, 128(2x64d)]
    def pack_qk(w):
        w2 = np.ascontiguousarray(w.transpose(1, 0, 2)).reshape(C, C)  # [c, h*64]
        w4 = w2.reshape(N_CC, 128, N_HP, 128).transpose(1, 2, 0, 3)   # [128c, hp, cc, 128]
        return np.ascontiguousarray(w4).reshape(128, -1).astype(proj_np)

    wq_h = pack_qk(Wq)
    wk_h = pack_qk(Wk)
    wv2 = np.ascontiguousarray(Wv.transpose(1, 0, 2)).reshape(C, C)    # [c, hd]
    wv_h = np.ascontiguousarray(
        wv2.reshape(N_CC, 128, C).transpose(1, 0, 2)).reshape(128, -1).astype(proj_np)
    wo_h = np.ascontiguousarray(
        Wo.reshape(N_CC, 128, C).transpose(1, 0, 2)).reshape(128, -1).astype(op_np)

    ii, jj = np.indices((128, 128))
    mask_h = (jj >= ii).astype(pv_np)   # [s, t]: valid when t >= s
    id_h = np.eye(128, dtype=op_np)

    in_maps = []
    for core in range(N_CORES):
        xs = x[core * B_LOC:(core + 1) * B_LOC]              # [8, 256, 512]
        # [128p, cc, b, t]: c = cc*128 + p
        xt = xs.transpose(2, 0, 1).reshape(N_CC, 128, B_LOC, T)
        xt = np.ascontiguousarray(xt.transpose(1, 0, 2, 3)).reshape(128, -1)
        in_maps.append({
            "xT": xt.astype(proj_np), "wq": wq_h, "wk": wk_h, "wv": wv_h,
            "wo": wo_h, "mask": mask_h, "ident": id_h,
        })
    return in_maps


DEFAULT_CFG = {"proj": "fp16", "scores": "fp16", "pv": "fp16", "outproj": "fp16"}

_NC_CACHE = {}


def run(x, Wq, Wk, Wv, Wo, cfg=None, trace=False):
    cfg = cfg or DEFAULT_CFG
    key = tuple(sorted(cfg.items()))
    if key not in _NC_CACHE:
        _NC_CACHE[key] = build_nc(cfg)
    nc = _NC_CACHE[key]
    in_maps = _prep_inputs(np.asarray(x), np.asarray(Wq), np.asarray(Wk),
                           np.asarray(Wv), np.asarray(Wo), cfg)
    res = run_bass_kernel_spmd(nc, in_maps, core_ids=list(range(N_CORES)),
                               trace=trace)
    y = np.concatenate([r["y"] for r in res.results], axis=0)
    return y, res


def kernel(x, Wq, Wk, Wv, Wo):
    y, _ = run(x, Wq, Wk, Wv, Wo)
    return y.astype(np.float32)


if __name__ == "__main__":
    import time
    t0 = time.time()
    nc = build_nc(DEFAULT_CFG)
    print(f"build+compile: {time.time()-t0:.1f}s")


# revision 45
# speedup vs baseline: 1.6438x; 1.6438x over previous
"""Trainium2 Bass kernel for causal multi-head attention (v3).

Problem: x[64,256,512] f32, Wq/Wk/Wv[8,512,64], Wo[512,512]
  q,k,v = einsum('btc,hcd->bhtd'); scores = q k^T / sqrt(512) (causal);
  out = softmax(scores) v; y = concat-heads(out) @ Wo.

Strategy: data-parallel over batch across 8 NeuronCores (8 batches/core,
no collectives), all-fp16 operands with fp32 PSUM accumulation.

v3 changes vs v2 (which used moving-P PV + reciprocal[1,512] + Pool
partition_broadcast + per-column normalize):
  - PV uses the masked P blocks as STATIONARY ([s,t] blocks) and V' as
    moving ([s, 65] with a ones column), producing out[t, h, 65] in PSUM
    with t on PARTITIONS; col 64 is the softmax denominator.
  - Normalize is then per-partition: one cheap [128,4] reciprocal and one
    broadcast tensor_mul per (tb, head-group) on the DVE, fusing the fp16
    downcast. No Pool broadcast, no 1-partition reciprocal.
  - out[t, hd] is transposed back to outT[hd, t] for the output
    projection with 4 PE transposes per (b, tb) (fp16 PSUM out) + 1 copy.
  - PSUM->SBUF copies distributed across Act/DVE/Pool engines (knobs).
  - y output DMAd as fp16 (upcast on host): halves output traffic.
"""
import numpy as np

import concourse.bass as bass
import concourse.tile as tile
import concourse.mybir as mybir
from concourse import bacc
from concourse.alu_op_type import AluOpType
from concourse.bass_utils import run_bass_kernel_spmd

F32 = mybir.dt.float32
F16 = mybir.dt.float16
BF16 = mybir.dt.bfloat16

N_CORES = 8
B, T, C = 64, 256, 512
H, DK = 8, 64
B_LOC = B // N_CORES        # 8 batches per core
N_HP = H // 2               # head pairs (2x64 packed on partitions)
N_CC = C // 128             # contraction chunks
SCALE = 1.0 / np.sqrt(np.float32(C))

_DT = {"fp32": F32, "fp16": F16, "bf16": BF16}


def build_nc(cfg, repeat=0, tune=None, bodies=1):
    """repeat=0: straight-line kernel. repeat=R>0: wrap the whole pipeline
    in a hardware For_i loop executed R times (for timing). bodies=N emits
    the pipeline N times inside the loop (probe for loop-boundary cost)."""
    tu = {"big": 2, "s": 2, "pv": 3, "tr": 1, "p": 6, "qk": 3, "vv": 2,
          "xt": 2, "ot": 2, "oT": 2, "rc": 4,
          "qkcop": "dve", "vcop": "act", "ycop": "act", "trcop": "dve",
          "mask_eng": "mix", "norm": "on", "mask": "on",
          "esplit": 0, "nd": 0, "xchunk": 0, "ymerge": 1}
    tu.update(tune or {})
    for k, v in (cfg or {}).items():
        if k.startswith("t_"):
            kk = k[2:]
            tu[kk] = int(v) if str(v).isdigit() else v
    proj_dt = _DT[cfg.get("proj", "fp16")]      # xT / Wq / Wk / Wv operands
    sc_dt = _DT[cfg.get("scores", "fp16")]      # QT / KT operands
    pv_dt = _DT[cfg.get("pv", "fp16")]          # P / V' operands
    op_dt = _DT[cfg.get("outproj", "fp16")]     # outT / Wo operands

    nc = bacc.Bacc("TRN2", target_bir_lowering=False, debug=False)

    xT_d = nc.dram_tensor("xT", [128, N_CC * B_LOC * T], proj_dt,
                          kind="ExternalInput").ap()
    wq_d = nc.dram_tensor("wq", [128, N_HP * N_CC * 128], proj_dt, kind="ExternalInput").ap()
    wk_d = nc.dram_tensor("wk", [128, N_HP * N_CC * 128], proj_dt, kind="ExternalInput").ap()
    wv_d = nc.dram_tensor("wv", [128, N_CC * C], proj_dt, kind="ExternalInput").ap()
    wo_d = nc.dram_tensor("wo", [128, N_CC * C], op_dt, kind="ExternalInput").ap()
    mask_d = nc.dram_tensor("mask", [128, 128], pv_dt, kind="ExternalInput").ap()
    id_d = nc.dram_tensor("ident", [128, 128], op_dt, kind="ExternalInput").ap()
    y_d = nc.dram_tensor("y", [B_LOC, T, C], F16, kind="ExternalOutput").ap()

    def copier(which):
        eng = tu[which]
        if eng == "act":
            return nc.scalar.copy
        if eng == "pool":
            return nc.gpsimd.tensor_copy
        return nc.vector.tensor_copy

    with tile.TileContext(nc) as tc:
        import contextlib
        ctx = contextlib.ExitStack()
        with ctx:
            const = ctx.enter_context(tc.tile_pool(name="const", bufs=1))
            xT_p = ctx.enter_context(tc.tile_pool(name="xT", bufs=tu["xt"]))
            qk_p = ctx.enter_context(tc.tile_pool(name="qk", bufs=tu["qk"]))
            vv_p = ctx.enter_context(tc.tile_pool(name="vv", bufs=tu["vv"]))
            p_p = ctx.enter_context(tc.tile_pool(name="pp", bufs=tu["p"]))
            rc_p = ctx.enter_context(tc.tile_pool(name="rc", bufs=tu["rc"]))
            ot_p = ctx.enter_context(tc.tile_pool(name="ot", bufs=tu["ot"]))
            oT_p = ctx.enter_context(tc.tile_pool(name="oT", bufs=tu["oT"]))
            y_p = ctx.enter_context(tc.tile_pool(name="yp", bufs=2))
            big_ps = ctx.enter_context(tc.tile_pool(name="big_ps", bufs=tu["big"], space="PSUM"))
            s_ps = ctx.enter_context(tc.tile_pool(name="s_ps", bufs=tu["s"], space="PSUM"))
            pv_ps = ctx.enter_context(tc.tile_pool(name="pv_ps", bufs=tu["pv"], space="PSUM"))
            tr_ps = big_ps if not tu["tr"] else ctx.enter_context(
                tc.tile_pool(name="tr_ps", bufs=tu["tr"], space="PSUM"))

            # persistent constants / weights; wq/wk first (gate QK(0) in
            # the single-shot path), then wv/mask, then late-need wo/ident
            wq = const.tile([128, N_HP, N_CC, 128], proj_dt)
            nc.sync.dma_start(wq[:], wq_d.rearrange("p (a b c) -> p a b c", a=N_HP, b=N_CC))
            wk = const.tile([128, N_HP, N_CC, 128], proj_dt)
            nc.sync.dma_start(wk[:], wk_d.rearrange("p (a b c) -> p a b c", a=N_HP, b=N_CC))
            wv = const.tile([128, N_CC, C], proj_dt)
            nc.sync.dma_start(wv[:], wv_d.rearrange("p (a b) -> p a b", a=N_CC))
            mask2 = const.tile([128, 2, 128], pv_dt)
            nc.sync.dma_start(mask2[:, 0, :], mask_d[:])
            nc.sync.dma_start(mask2[:, 1, :], mask_d[:])
            ident = const.tile([128, 128], op_dt)
            nc.sync.dma_start(ident[:], id_d[:])
            wo = const.tile([128, N_CC, C], op_dt)
            nc.sync.dma_start(wo[:], wo_d.rearrange("p (a b) -> p a b", a=N_CC))
            if tu["mask"] == "bias":
                # additive pre-exp causal mask, injected via PE: each diag
                # S block's accumulation group starts with ident.T @ mrow
                # (= -6e4 on masked, 0 on valid), then the score matmul
                # accumulates on top. exp output is already masked, so PV
                # no longer waits on a post-exp DVE mask multiply.
                mrow = const.tile([128, 128], pv_dt)
                nc.vector.tensor_scalar(
                    mrow[:], mask2[:, 0, :], 6.0e4, 6.0e4,
                    AluOpType.mult, AluOpType.subtract)

            lp = nc.allow_low_precision(
                reason="fp16 operands with fp32 PSUM accumulation; "
                       "rel-err budget 2e-2")

            def body():
                # ---- load xT for all batches: [128, cc, b, t]; chunked
                # along cc (contiguous 4KB/partition segments) so the first
                # QK accumulation starts after 1/4 of the data arrives and
                # the load pipelines across For_i iterations ----
                xT = xT_p.tile([128, N_CC, B_LOC, T], proj_dt, tag="xT")
                xT_src = xT_d.rearrange("p (a b t) -> p a b t", a=N_CC, b=B_LOC)
                if tu["xchunk"]:
                    for cc in range(N_CC):
                        nc.sync.dma_start(xT[:, cc, :, :], xT_src[:, cc, :, :])
                else:
                    nc.sync.dma_start(xT[:], xT_src)

                # Filler-queue software pipeline: attention head-steps of
                # batch b interleave ready PE chunk-work from the future
                # (QK of pair+1, V of b+1, OP of b-1) so the in-order PE
                # queue always has independent work during chain stalls.
                from collections import deque
                filler = deque()
                state = {}
                qk_tiles = {}
                vv_tiles = {}
                outT_tiles = {}

                def make_qk_fillers(pair):
                    qt = qk_p.tile([128, N_HP, 2, T], sc_dt, tag="qt",
                                   name=f"qt{pair}")
                    kt = qk_p.tile([128, N_HP, 2, T], sc_dt, tag="kt",
                                   name=f"kt{pair}")
                    qk_tiles[pair] = (qt, kt)
                    chunks = []
                    for (w, dst) in ((wq, qt), (wk, kt)):
                        for hp in range(N_HP):
                            def chunk(w=w, dst=dst, hp=hp, pair=pair):
                                ps = big_ps.tile([128, 512], F32, tag="big")
                                for cc in range(N_CC):
                                    nc.tensor.matmul(
                                        ps[:], w[:, hp, cc, :],
                                        xT[:, cc, 2 * pair:2 * pair + 2, :].rearrange("p a t -> p (a t)"),
                                        start=(cc == 0), stop=(cc == N_CC - 1))
                                dstap = dst[:, hp, :, :].rearrange("p a t -> p (a t)")
                                copier("qkcop")(dstap, ps[:])
                            chunks.append(chunk)
                    return chunks

                def make_v_fillers(b):
                    vvt = vv_p.tile([128, 2, H, DK + 1], pv_dt, tag="vv",
                                    name=f"vv{b}")
                    vv_tiles[b] = vvt
                    chunks = []
                    for sc in range(2):
                        def chunk(sc=sc, vvt=vvt, b=b):
                            if sc == 0:
                                nc.vector.memset(vvt[:, :, :, DK:DK + 1], 1.0)
                            ps = big_ps.tile([128, 512], F32, tag="big")
                            for cc in range(N_CC):
                                nc.tensor.matmul(
                                    ps[:], xT[:, cc, b, bass.ts(sc, 128)], wv[:, cc, :],
                                    start=(cc == 0), stop=(cc == N_CC - 1))
                            copier("vcop")(
                                vvt[:, sc, :, 0:DK],
                                ps[:].rearrange("p (h d) -> p h d", h=H))
                        chunks.append(chunk)
                    return chunks

                def make_op_fillers(b):
                    outT = outT_tiles[b]
                    chunks = []
                    if tu["ymerge"]:
                        yt = y_p.tile([128, 2, C], F16, tag="yt", name=f"yt{b}")
                    for tb in range(2):
                        def chunk(tb=tb, outT=outT, b=b):
                            ps = big_ps.tile([128, 512], F32, tag="big")
                            for cc in range(N_CC):
                                nc.tensor.matmul(
                                    ps[:], outT[:, cc, tb, :], wo[:, cc, :],
                                    start=(cc == 0), stop=(cc == N_CC - 1))
                            if tu["ymerge"]:
                                copier("ycop")(yt[:, tb, :], ps[:])
                                if tb == 1:
                                    nc.sync.dma_start(
                                        y_d[b].rearrange("(a q) c -> q a c", a=2),
                                        yt[:])
                            else:
                                yt1 = y_p.tile([128, C], F16, tag="yt")
                                copier("ycop")(yt1[:], ps[:])
                                nc.sync.dma_start(y_d[b, bass.ts(tb, 128), :], yt1[:])
                        chunks.append(chunk)
                    return chunks

                def emit_S(b2, h, qt, kt):
                    hp, lo = h // 2, (h % 2) * DK
                    qs = qt[lo:lo + DK, hp, b2, :]
                    ks = kt[lo:lo + DK, hp, b2, :]
                    sps = s_ps.tile([128, 384], F32, tag="s")
                    if tu["mask"] == "bias":
                        # col layout [tri0 | tri1 | full]; diag block groups
                        # start with the additive-mask matmul (consts only,
                        # so PE can run it before qs/ks are even ready)
                        nc.tensor.matmul(sps[:, 0:128], ident[:], mrow[:],
                                         start=True, stop=False)
                        nc.tensor.matmul(sps[:, 0:128], ks[:, 0:128],
                                         qs[:, 0:128], start=False, stop=True)
                        nc.tensor.matmul(sps[:, 128:256], ident[:], mrow[:],
                                         start=True, stop=False)
                        nc.tensor.matmul(sps[:, 128:256], ks[:, 128:256],
                                         qs[:, 128:256], start=False, stop=True)
                        nc.tensor.matmul(sps[:, 256:384], ks[:, 0:128],
                                         qs[:, 128:256], start=True, stop=True)
                    else:
                        nc.tensor.matmul(sps[:, 0:256], ks[:, 0:128], qs[:],
                                         start=True, stop=True)
                        nc.tensor.matmul(sps[:, 256:384], ks[:, 128:256],
                                         qs[:, 128:256], start=True, stop=True)
                    pt = p_p.tile([128, 3, 128], pv_dt, tag="p")
                    p = pt[:].rearrange("q a b -> q (a b)")
                    Exp = mybir.ActivationFunctionType.Exp

                    def domask(pm, m):
                        if tu["mask_eng"] == "pool" or (
                                tu["mask_eng"] == "mix" and h % 2 == 1):
                            nc.gpsimd.tensor_mul(pm, pm, m)
                        else:
                            nc.vector.tensor_mul(pm, pm, m)

                    if tu["esplit"]:
                        # diag blocks first: unblocks mask+PV(A,C) sooner
                        sps3 = sps[:].rearrange("p (a b) -> p a b", a=3)
                        nc.scalar.activation(pt[:, 0:3:2, :], sps3[:, 0:3:2, :],
                                             Exp, bias=0.0, scale=float(SCALE))
                        if tu["mask"] != "off":
                            domask(pt[:, 0:3:2, :], mask2[:])
                        nc.scalar.activation(pt[:, 1, :], sps[:, 128:256],
                                             Exp, bias=0.0, scale=float(SCALE))
                    else:
                        nc.scalar.activation(p, sps[:], Exp,
                                             bias=0.0, scale=float(SCALE))
                        if tu["mask"] == "on":
                            domask(pt[:, 0:3:2, :], mask2[:])
                    state[h] = {"pt": pt}

                def emit_PV(b, h, vvt, pvts):
                    pt = state[h]["pt"]
                    g, slot = h // 4, h % 4
                    if slot == 0:
                        pvts[g] = (
                            pv_ps.tile([128, 4, DK + 1], F32, tag="pv",
                                       name=f"pv{b}t0g{g}"),
                            pv_ps.tile([128, 4, DK + 1], F32, tag="pv",
                                       name=f"pv{b}t1g{g}"),
                        )
                    p0, p1 = pvts[g]
                    # block indices in pt: mask=="bias" layout [tri0|tri1|full]
                    # vs default [tri0|full|tri1]
                    iB, iC = (2, 1) if tu["mask"] == "bias" else (1, 2)
                    nc.tensor.matmul(p0[:, slot, :], pt[:, 0, :], vvt[:, 0, h, :],
                                     start=True, stop=True)
                    nc.tensor.matmul(p1[:, slot, :], pt[:, iB, :], vvt[:, 0, h, :],
                                     start=True, stop=False)
                    nc.tensor.matmul(p1[:, slot, :], pt[:, iC, :], vvt[:, 1, h, :],
                                     start=False, stop=True)
                    state.pop(h)

                def emit_norm(b, g, outn, pvts):
                    for tb in range(2):
                        pv = pvts[g][tb]
                        if tu["norm"] == "on":
                            rec = rc_p.tile([128, 4, 1], pv_dt, tag="rec")
                            nc.vector.reciprocal(rec[:], pv[:, :, DK:DK + 1])
                            nc.vector.tensor_mul(
                                outn[:, tb, bass.ts(g, 4), :], pv[:, :, 0:DK],
                                rec[:].broadcast_to([128, 4, DK]))
                        else:  # timing ablation: skip normalize chain
                            nc.vector.tensor_copy(
                                outn[:, tb, bass.ts(g, 4), :], pv[:, :, 0:DK])

                def emit_tr(b, outn, trst, ks):
                    # transpose head-blocks ks (0,1 ready after g0 norm;
                    # 2,3 after g1) for both t-blocks into one fp16 psum
                    # tile (both tbs fit in a single bank)
                    if trst["tps"] is None:
                        trst["tps"] = tr_ps.tile(
                            [128, 2, N_CC, 128], op_dt,
                            tag="big" if tr_ps is big_ps else "tr",
                            name=f"tr{b}")
                    tps = trst["tps"]
                    on2 = outn[:].rearrange("p a b c -> p a (b c)")
                    for tb in range(2):
                        for k in ks:
                            nc.tensor.transpose(
                                tps[:, tb, k, :],
                                on2[:, tb, bass.ts(k, 128)], ident[:])

                def emit_trcopy(b, trst):
                    outT = oT_p.tile([128, N_CC, 2, 128], op_dt, tag="outT",
                                     name=f"ot{b}")
                    outT_tiles[b] = outT
                    copier("trcop")(
                        outT[:], trst["tps"][:].rearrange("p a b c -> p b a c"))

                # bootstrap: QK(0) and V(0) emitted inline
                for f in make_qk_fillers(0):
                    f()
                for f in make_v_fillers(0):
                    f()

                for pair in range(B_LOC // 2):
                    qt, kt = qk_tiles[pair]
                    for b2 in range(2):
                        b = 2 * pair + b2
                        outn = ot_p.tile([128, 2, H, DK], pv_dt, tag="outn",
                                         name=f"on{b}")
                        pvts = {}
                        trst = {"tps": None}
                        if b2 == 1 and pair + 1 < B_LOC // 2:
                            filler.extend(make_qk_fillers(pair + 1))
                        if b + 1 < B_LOC:
                            filler.extend(make_v_fillers(b + 1))
                        if b - 1 >= 0:
                            filler.extend(make_op_fillers(b - 1))

                        vvt = vv_tiles[b]
                        for h in range(H):
                            emit_S(b2, h, qt, kt)
                            if tu["nd"] == 0:
                                ndrain = 2 if len(filler) > H - h else 1
                            elif tu["nd"] == 1:
                                ndrain = 1
                            elif tu["nd"] == 2:
                                ndrain = 2
                            else:  # front-loaded
                                ndrain = 2 if h < 4 else 1
                            for _ in range(ndrain):
                                if filler:
                                    filler.popleft()()
                            if h >= 1:
                                emit_PV(b, h - 1, vvt, pvts)
                            if h == 4:
                                emit_norm(b, 0, outn, pvts)
                            if h == 5:
                                emit_tr(b, outn, trst, (0, 1))
                        emit_PV(b, H - 1, vvt, pvts)
                        emit_norm(b, 1, outn, pvts)
                        emit_tr(b, outn, trst, (2, 3))
                        emit_trcopy(b, trst)
                while filler:
                    filler.popleft()()
                for f in make_op_fillers(B_LOC - 1):
                    f()

            with lp:
                if repeat:
                    with tc.For_i(0, repeat, 1):
                        for _ in range(bodies):
                            body()
                else:
                    for _ in range(bodies):
                        body()

    nc.compile()
    return nc


def _prep_inputs(x, Wq, Wk, Wv, Wo, cfg):
    """Host-side reshapes/casts. Returns per-core input maps."""
    import ml_dtypes

    def npdt(key):
        s = cfg.get(key, "fp16")
        return {"fp32": np.float32, "fp16": np.float16,
                "bf16": ml_dtypes.bfloat16}[s]

    proj_np, sc_np, pv_np, op_np = (npdt(k) for k in
                                    ("proj", "scores", "pv", "outproj"))

    # weights: head-pair stationary blocks [128c, hp, cc, 128(2x64d)]
    def pack_qk(w):
        w2 = np.ascontiguousarray(w.transpose(1, 0, 2)).reshape(C, C)  # [c, h*64]
        w4 = w2.reshape(N_CC, 128, N_HP, 128).transpose(1, 2, 0, 3)   # [128c, hp, cc, 128]
        return np.ascontiguousarray(w4).reshape(128, -1).astype(proj_np)

    wq_h = pack_qk(Wq)
    wk_h = pack_qk(Wk)
    wv2 = np.ascontiguousarray(Wv.transpose(1, 0, 2)).reshape(C, C)    # [c, hd]
    wv_h = np.ascontiguousarray(
        wv2.reshape(N_CC, 128, C).transpose(1, 0, 2)).reshape(128, -1).astype(proj_np)
    wo_h = np.ascontiguousarray(
        Wo.reshape(N_CC, 128, C).transpose(1, 0, 2)).reshape(128, -1).astype(op_np)

    ii, jj = np.indices((128, 128))
    mask_h = (jj >= ii).astype(pv_np)   # [s, t]: valid when t >= s
    id_h = np.eye(128, dtype=op_np)

    in_maps = []
    for core in range(N_CORES):
        xs = x[core * B_LOC:(core + 1) * B_LOC]              # [8, 256, 512]
        # [128p, cc, b, t]: c = cc*128 + p
        xt = xs.transpose(2, 0, 1).reshape(N_CC, 128, B_LOC, T)
        xt = np.ascontiguousarray(xt.transpose(1, 0, 2, 3)).reshape(128, -1)
        in_maps.append({
            "xT": xt.astype(proj_np), "wq": wq_h, "wk": wk_h, "wv": wv_h,
            "wo": wo_h, "mask": mask_h, "ident": id_h,
        })
    return in_maps


DEFAULT_CFG = {"proj": "fp16", "scores": "fp16", "pv": "fp16", "outproj": "fp16"}

_NC_CACHE = {}


def run(x, Wq, Wk, Wv, Wo, cfg=None, trace=False):
    cfg = cfg or DEFAULT_CFG
    key = tuple(sorted(cfg.items()))
    if key not in _NC_CACHE:
        _NC_CACHE[key] = build_nc(cfg)
    nc = _NC_CACHE[key]
    in_maps = _prep_inputs(np.asarray(x), np.asarray(Wq), np.asarray(Wk),
                           np.asarray(Wv), np.asarray(Wo), cfg)
    res = run_bass_kernel_spmd(nc, in_maps, core_ids=list(range(N_CORES)),
                               trace=trace)
    y = np.concatenate([r["y"] for r in res.results], axis=0)
    return y, res


def kernel(x, Wq, Wk, Wv, Wo):
    y, _ = run(x, Wq, Wk, Wv, Wo)
    return y.astype(np.float32)


if __name__ == "__main__":
    import time
    t0 = time.time()
    nc = build_nc(DEFAULT_CFG)
    print(f"build+compile: {time.time()-t0:.1f}s")
